# revision 3
# baseline (speedup 1.0000x reference)
"""Trainium2 Bass kernel for nn_LogReg_30193620091430 — v11 (final).

Per core (data-parallel over 8 cores, 1250 graphs / 125k nodes each):

  Layout exploits 100 nodes/graph * 1 KiB/node: group q of 64 graphs is
  [128 partition-lines x 50 rows], line p = rows 6400q + 50p .. +49.
  Each group loads as TWO 25-row halves (25 KiB contiguous
  descriptors).  128-descriptor dma_starts are mandatory: the DGE deals
  equal chunks of descriptors to <=16 engines (chunk = ndesc/nengines,
  equal split), so 128 descs -> 16 engines x 8, while e.g. 68 descs
  would collapse to 4 engines x 17.  The odd 34-graph group (68 lines)
  is therefore loaded as partition-split (64+4)-line dma_starts, and is
  processed FIRST so the stream ends on a full group whose PE tail is
  short.  Halves alternate between the two HWDGE queues; a 7-buffer
  pool gives 3.5 groups of lookahead so the PE (which runs ~12 us/group
  at mid p-state) never back-pressures the DMA queues.

  stage 1: segment-sum via f32r matmuls with a single fixed stationary
  stat[p, g] = (p//2 == g): 50 accumulating matmuls per group (moving =
  [np, 256] row slice, j<25 from half A so the PE starts at half-group
  latency) -> PSUM pooled [64, 256].

  stage 2 per group (software-pipelined one group deep so the strict-
  FIFO PE queue never stalls): PSUM->SBUF copy (vector), PE transpose
  of both 128-feature halves, FC with host-pretransposed W + bias via
  ones-row matmul, PReLU as one scalar_tensor_tensor max(a*x, x)
  (valid for a <= 1).  Output stores ride the gpsimd SWDGE queue per
  group so they overlap with the stream and never block seq loads.
"""
import numpy as np

NUM_GRAPHS = 10000
NODES_PER_GRAPH = 100
FT_IN = 256
NB_CLASSES = 128
N_CORES = 8

G_CORE = NUM_GRAPHS // N_CORES            # 1250 graphs per core
N_CORE = G_CORE * NODES_PER_GRAPH         # 125000 nodes per core
GB = 64                                   # graphs per group / PSUM tile
NGROUPS = (G_CORE + GB - 1) // GB         # 20 groups (19 full + one of 34)
RP = NODES_PER_GRAPH // 2                 # 50 rows per partition-line
RH = RP // 2                              # 25 rows per half

_CACHE = {}


def _build_module():
    import concourse.bacc as bacc
    import concourse.mybir as mybir
    from concourse.tile import TileContext

    F32 = mybir.dt.float32
    F32R = mybir.dt.float32r
    BF16 = mybir.dt.bfloat16
    U16 = mybir.dt.uint16
    F = FT_IN
    C = NB_CLASSES

    nc = bacc.Bacc(None, target_bir_lowering=False)
    seq = nc.dram_tensor("seq", [N_CORE, F], F32, kind="ExternalInput")
    # pack1[p]: [0:GB) stat | [GB:GB+2C) wt | row0 [GB+2C:GB+3C) b,
    # [GB+3C:GB+3C+GB) ones
    P1 = GB + 3 * C + GB
    pk1 = nc.dram_tensor("pk1", [128, P1], F32, kind="ExternalInput")
    # pack2[g]: [0:GB) ident | [GB] a
    pk2 = nc.dram_tensor("pk2", [GB, GB + 1], F32, kind="ExternalInput")
    pk3 = nc.dram_tensor("pk3", [128, GB], U16, kind="ExternalInput")
    out = nc.dram_tensor("out", [G_CORE, C], F32, kind="ExternalOutput")

    # process the odd 34-graph group first; end on a full group
    order = [NGROUPS - 1] + list(range(NGROUPS - 1))

    with TileContext(nc) as tc:
        with (
            tc.tile_pool(name="const", bufs=1) as cpool,
            tc.tile_pool(name="seqp", bufs=14) as seqp,
            tc.tile_pool(name="s2", bufs=3) as s2,
            tc.tile_pool(name="ps1", bufs=4, space="PSUM") as ps1,
            tc.tile_pool(name="pst", bufs=2, space="PSUM") as pst,
            tc.tile_pool(name="ps2", bufs=2, space="PSUM") as ps2,
        ):
            pk1_t = cpool.tile([128, P1], F32R)
            nc.gpsimd.dma_start(pk1_t[:, :], pk1[:, :].bitcast(F32R))
            pk2_t = cpool.tile([GB, GB + 1], F32)
            nc.gpsimd.dma_start(pk2_t[:, :], pk2[:, :])
            statb_t = cpool.tile([128, GB], BF16)
            nc.gpsimd.dma_start(statb_t[:, :], pk3[:, :].bitcast(BF16))
            stat_t = pk1_t[:, 0:GB]
            wt_sb = pk1_t[:, GB:GB + 2 * C]
            b_sb = pk1_t[0:1, GB + 2 * C:GB + 3 * C]
            ones_t = pk1_t[0:1, GB + 3 * C:GB + 3 * C + GB]
            ident_t = pk2_t[:, 0:GB]
            a_col = pk2_t[:, GB:GB + 1]

            qcnt = 0

            def eng():
                nonlocal qcnt
                qcnt += 1
                return nc.sync if qcnt % 2 == 1 else nc.scalar

            # group q half h: line p <- rows 6400q + 50p + 25h .. +24
            # (25 KiB contiguous per line -> one descriptor)
            halves = {}
            for q in order:
                ng = min(GB, G_CORE - GB * q)
                np_ = 2 * ng
                src = seq[GB * NODES_PER_GRAPH * q:
                          GB * NODES_PER_GRAPH * q + RP * np_, :].rearrange(
                    "(p s r) f -> p s r f", s=2, r=RH)
                pair = []
                for h in range(2):
                    sq = seqp.tile([128, RH * F], BF16)
                    # partition-split so every dma_start has a descriptor
                    # count that fans out to all 16 engines (68 -> 64+4);
                    # gpsimd SWDGE casts f32 -> bf16 in flight
                    cuts = (0, np_) if np_ % 128 == 0 else (0, 64, np_)
                    for c0, c1 in zip(cuts, cuts[1:]):
                        nc.gpsimd.dma_start(
                            sq[c0:c1, :].rearrange("p (r f) -> p r f", r=RH),
                            src[c0:c1, h, :, :],
                        )
                    pair.append(sq)
                halves[q] = (ng, np_, pair)

            # Stage 2 is software-pipelined one group deep: the previous
            # group's PE ops are emitted inside/after the current group's
            # stage-1 matmuls so the strict-FIFO PE queue never stalls on
            # DVE dependencies.
            pend = {}

            def s2_transposes(qp):
                st = pend[qp]
                st["pt_sb"] = s2.tile([128, 2 * GB], F32R, name="pt_sb")
                for h in range(2):
                    tp = pst.tile([128, 512], F32, tag="tp", name="tp")
                    nc.tensor.transpose(
                        tp[:, :st["ng"]],
                        st["pooled_sb"][:st["ng"], 128 * h:128 * (h + 1)],
                        ident_t[:st["ng"], :st["ng"]])
                    nc.vector.tensor_copy(
                        st["pt_sb"][:, GB * h:GB * h + st["ng"]],
                        tp[:, :st["ng"]])

            def s2_finish(qp):
                st = pend.pop(qp)
                ng_, pt_sb = st["ng"], st["pt_sb"]
                ret_ps = ps2.tile([GB, 512], F32, tag="ret", name="ret_ps")
                nc.tensor.matmul(ret_ps[:ng_, :C], ones_t[:1, :ng_],
                                 b_sb[:1, :], start=True, stop=False)
                for h in range(2):
                    nc.tensor.matmul(
                        ret_ps[:ng_, :C],
                        pt_sb[:, GB * h:GB * h + ng_],
                        wt_sb[:, C * h:C * (h + 1)],
                        start=False, stop=(h == 1),
                    )
                ret_sb = s2.tile([GB, C], F32, name="ret_sb")
                nc.vector.tensor_copy(ret_sb[:ng_, :], ret_ps[:ng_, :C])
                ret_out = s2.tile([GB, C], F32, name="ret_out")
                nc.vector.scalar_tensor_tensor(
                    ret_out[:ng_, :], ret_sb[:ng_, :],
                    a_col[:ng_, 0:1], ret_sb[:ng_, :],
                    op0=mybir.AluOpType.mult, op1=mybir.AluOpType.max,
                )
                eng().dma_start(out[GB * qp:GB * qp + ng_, :],
                                ret_out[:ng_, :])

            prev = None
            for q in order:
                ng, np_, pair = halves[q]
                pooled_ps = ps1.tile([GB, 512], F32)
                for j in range(RP):
                    sq = pair[j // RH]
                    jj = j % RH
                    nc.tensor.matmul(
                        pooled_ps[:, :F],
                        statb_t[:np_, :],
                        sq[:np_, jj * F:(jj + 1) * F],
                        start=(j == 0), stop=(j == RP - 1),
                    )
                    if j == 13 and prev in pend:
                        s2_transposes(prev)

                if prev in pend:
                    s2_finish(prev)
                pooled_sb = s2.tile([GB, F], F32)
                nc.vector.tensor_copy(pooled_sb[:ng, :], pooled_ps[:ng, :F])
                pend[q] = {"ng": ng, "pooled_sb": pooled_sb}
                prev = q

            s2_transposes(prev)
            s2_finish(prev)

    nc.finalize()
    return nc


def prepare_in_maps(seq, graph_len, W, b, prelu_a):
    seq = np.ascontiguousarray(np.asarray(seq, dtype=np.float32))
    W = np.asarray(W, dtype=np.float32)
    C = NB_CLASSES
    # wt[p, 128h + c] = W[c, 128h + p]  (pretransposed halves of W)
    wt = np.empty((128, 2 * C), dtype=np.float32)
    for h in range(2):
        wt[:, C * h:C * (h + 1)] = W[:, 128 * h:128 * (h + 1)].T
    P1 = GB + 3 * C + GB
    pk1 = np.zeros((128, P1), dtype=np.float32)
    pk1[np.arange(128), np.arange(128) // 2] = 1.0        # stat
    pk1[:, GB:GB + 2 * C] = wt
    pk1[0, GB + 2 * C:GB + 3 * C] = np.asarray(b, dtype=np.float32)
    pk1[0, GB + 3 * C:GB + 3 * C + GB] = 1.0              # ones
    pk2 = np.zeros((GB, GB + 1), dtype=np.float32)
    pk2[:, :GB] = np.eye(GB, dtype=np.float32)
    pk2[:, GB] = np.float32(np.asarray(prelu_a))
    pk3 = np.zeros((128, GB), dtype=np.uint16)
    pk3[np.arange(128), np.arange(128) // 2] = 0x3F80    # bf16 1.0

    shards = seq.reshape(N_CORES, N_CORE, FT_IN)
    return [
        {"seq": shards[i], "pk1": pk1, "pk2": pk2, "pk3": pk3}
        for i in range(N_CORES)
    ]


def kernel(seq, graph_len, W, b, prelu_a):
    from concourse.bass_utils import run_bass_kernel_spmd

    if "nc" not in _CACHE:
        _CACHE["nc"] = _build_module()
    nc = _CACHE["nc"]

    in_maps = prepare_in_maps(seq, graph_len, W, b, prelu_a)
    res = run_bass_kernel_spmd(nc, in_maps, core_ids=list(range(N_CORES)))
    return np.concatenate([r["out"] for r in res.results], axis=0)
